# revision 1
# baseline (speedup 1.0000x reference)
"""Trainium2 Bass kernel for the CSCG batched masked HMM forward pass.

Problem: for each of B=8 padded observation sequences, run a log-space HMM
forward recurrence restricted to 512-state clone blocks selected by
consecutive observation pairs, and read log P(obs) at true_len-1.

Strategy (one sequence per NeuronCore, 8 cores):
  * Work in linear space with a scalar log-offset instead of logsumexp:
        v_{t+1} = (v_t @ exp(blk_t)) * 2^e_k   (occasionally / c, tracked in L)
    The 512x512 linear matvec runs on the TensorEngine as 16 PSUM-accumulated
    (K=128, M=128, N=1) matmuls whose input/output layout is identical
    ([128 partitions = low 7 bits of state, 4 free = high 2 bits]), so the
    serial chain needs no transposes.
  * Phase 1 precomputes exp(log_T)*S in fp8e4 into a block-major DRAM scratch
    (256 blocks of 512x512 -> 128 rows x 2KB each), cutting steady-state
    streaming traffic 4x vs f32.
  * Per step, one indirect DMA gathers the 256KB block for observation pair
    (o_{t-1}, o_t) using a host-precomputed row-index table.
  * Steps past true_len-1 multiply by a constant pad block that preserves
    sum(v) exactly, so all cores run a uniform step count and the final
    readout log(sum(v)) + L equals the value at true_len-1.
"""

import math
from contextlib import ExitStack

import numpy as np
import ml_dtypes

N_OBS = 16
C = 512
N_STATES = N_OBS * C  # 8192
B = 8
T = 1024
N_CORES = 8
PAD_BLOCK = N_OBS * N_OBS  # index of the constant pad block
N_TABLE_ROWS = (PAD_BLOCK + 1) * 128  # 33024 gather rows of 2048 bytes


def _build_bass(n_steps: int, ln_S: float, e_k: int, renorm_every: int,
                renorm_defer: int, blk_bufs: int = 6, repeat_p1: int = 1,
                repeat_p2: int = 1):
    import concourse.bass as bass
    import concourse.tile as tile
    from concourse import bacc, mybir

    fp8 = mybir.dt.float8e4
    bf16 = mybir.dt.bfloat16
    f32 = mybir.dt.float32
    i32 = mybir.dt.int32
    Act = mybir.ActivationFunctionType

    kappa = 2.0 ** (-9 - e_k)  # pad-block entry; exact in fp8e4 for e_k in [-16, 0]
    k_copy = 2.0 ** e_k

    nc = bacc.Bacc(None, target_bir_lowering=False)
    logT_in = nc.dram_tensor("log_T", [N_STATES, N_STATES], f32, kind="ExternalInput")
    offs_in = nc.dram_tensor("offs", [128, max(n_steps, 1)], i32, kind="ExternalInput")
    v0_in = nc.dram_tensor("v0", [128, 4], bf16, kind="ExternalInput")
    out_t = nc.dram_tensor("out", [1, 1], f32, kind="ExternalOutput")
    p_out = nc.dram_tensor("p_out", [128, 4], f32, kind="ExternalOutput")
    L_out = nc.dram_tensor("L_out", [1, 1], f32, kind="ExternalOutput")
    scratch = nc.dram_tensor("scratch", [N_TABLE_ROWS, 2048], fp8, kind="Internal")

    with ExitStack() as ctx:
        tc = ctx.enter_context(tile.TileContext(nc))

        # ---------------- Phase 1: exp(log_T)*S -> fp8 block-major scratch ----
        pin = ctx.enter_context(tc.tile_pool(name="pin", bufs=2))
        pf8 = ctx.enter_context(tc.tile_pool(name="pf8", bufs=2))

        pad_tile = pf8.tile([128, 2048], fp8, tag="pad")
        nc.vector.memset(pad_tile[:], kappa)
        nc.sync.dma_start(
            scratch[PAD_BLOCK * 128:(PAD_BLOCK + 1) * 128, :], pad_tile[:]
        )

        bias_tile = pf8.tile([128, 1], f32, tag="bias")
        nc.vector.memset(bias_tile[:], float(ln_S))

        for _p1 in range(repeat_p1):
            for rt in range(N_STATES // 128):
                tin = pin.tile([128, N_STATES], f32, tag="tin")
                nc.sync.dma_start(tin[:], logT_in[rt * 128:(rt + 1) * 128, :])
                tf8 = pf8.tile([128, N_STATES], fp8, tag="tf8")
                nc.scalar.activation(tf8[:], tin[:], Act.Exp,
                                     bias=bias_tile[:, 0:1], scale=1.0)
                p_blk, i_hi = divmod(rt, 4)
                # dest rows (p_blk*16 + c)*128 + i_lo, byte cols i_hi*512 + j
                dst = scratch[p_blk * 2048:(p_blk + 1) * 2048,
                              i_hi * 512:(i_hi + 1) * 512]
                dst = dst.rearrange("(c i) j -> i c j", c=16)
                src = tf8[:].rearrange("i (c j) -> i c j", c=16)
                nc.sync.dma_start(dst, src)

        # ---------------- Phase 2: the recurrence ----------------------------
        pconst = ctx.enter_context(tc.tile_pool(name="pconst", bufs=1))
        pblk = ctx.enter_context(tc.tile_pool(name="pblk", bufs=blk_bufs))
        pp = ctx.enter_context(tc.tile_pool(name="pp", bufs=3))
        pscale = ctx.enter_context(tc.tile_pool(name="pscale", bufs=2))
        psmall = ctx.enter_context(tc.tile_pool(name="psmall", bufs=2))
        ps_v = ctx.enter_context(tc.tile_pool(name="ps_v", bufs=4, space="PSUM"))
        ps_c = ctx.enter_context(tc.tile_pool(name="ps_c", bufs=2, space="PSUM"))
        ps_b = ctx.enter_context(tc.tile_pool(name="ps_b", bufs=2, space="PSUM"))

        offs_sb = pconst.tile([128, max(n_steps, 1)], i32)
        nc.sync.dma_start(offs_sb[:], offs_in[:])

        ones_col = pconst.tile([128, 1], bf16)
        nc.vector.memset(ones_col[:], 1.0)
        ones_row = pconst.tile([1, 128], f32)
        nc.vector.memset(ones_row[:], 2.0 ** (-e_k))
        L_tile = pconst.tile([1, 1], f32)
        nc.vector.memset(L_tile[:], 0.0)

        for _p2 in range(repeat_p2):
            p_cur = pp.tile([128, 4], bf16, tag="p")
            nc.sync.dma_start(p_cur[:], v0_in[:])

            pending_scale = {}  # apply_step -> scale AP [128,1] with 2^e_k / c

            for k in range(1, n_steps + 1):
                blk = pblk.tile([128, 2048], fp8, tag="blk")
                nc.gpsimd.indirect_dma_start(
                    out=blk[:],
                    out_offset=None,
                    in_=scratch[:],
                    in_offset=bass.IndirectOffsetOnAxis(
                        ap=offs_sb[:, k - 1:k], axis=0),
                )

                psum = ps_v.tile([128, 4], f32, tag="v")
                for j_hi in range(4):
                    for i_hi in range(4):
                        nc.tensor.matmul(
                            out=psum[:, j_hi:j_hi + 1],
                            lhsT=blk[:, i_hi * 512 + j_hi * 128:
                                     i_hi * 512 + (j_hi + 1) * 128],
                            rhs=p_cur[:, i_hi:i_hi + 1],
                            start=(i_hi == 0),
                            stop=(i_hi == 3),
                        )

                p_next = pp.tile([128, 4], bf16, tag="p")
                if k in pending_scale:
                    nc.vector.tensor_scalar_mul(p_next[:], psum[:],
                                                pending_scale.pop(k))
                else:
                    nc.vector.tensor_scalar_mul(p_next[:], psum[:], k_copy)
                p_cur = p_next

                # Deferred global renorm: measure sum(p) now, apply a few
                # steps later so the reciprocal/broadcast chain stays off the
                # critical path; L accumulates log(c) to keep the readout
                # invariant.
                if renorm_every and k % renorm_every == 0 \
                        and k + renorm_defer <= n_steps:
                    c_ps = ps_c.tile([1, 4], f32, tag="c")
                    nc.tensor.matmul(out=c_ps[:], lhsT=ones_col[:],
                                     rhs=p_cur[:], start=True, stop=True)
                    c_sb = psmall.tile([1, 1], f32, tag="c_sb")
                    nc.vector.reduce_sum(c_sb[:], c_ps[:],
                                         axis=mybir.AxisListType.X)
                    bc_ps = ps_b.tile([128, 1], f32, tag="bc")
                    nc.tensor.matmul(out=bc_ps[:], lhsT=ones_row[:],
                                     rhs=c_sb[:], start=True, stop=True)
                    scale_sb = pscale.tile([128, 1], f32, tag="scale")
                    nc.vector.reciprocal(scale_sb[:], bc_ps[:])
                    lnc = psmall.tile([1, 1], f32, tag="lnc")
                    nc.scalar.activation(lnc[:], c_sb[:], Act.Ln)
                    nc.vector.tensor_add(L_tile[:], L_tile[:], lnc[:])
                    pending_scale[k + renorm_defer] = scale_sb[:, 0:1]

        # ---------------- Readout: log(sum(v)) + L ---------------------------
        f_ps = ps_c.tile([1, 4], f32, tag="c")
        nc.tensor.matmul(out=f_ps[:], lhsT=ones_col[:], rhs=p_cur[:],
                         start=True, stop=True)
        s_sb = psmall.tile([1, 1], f32, tag="c_sb")
        nc.vector.reduce_sum(s_sb[:], f_ps[:], axis=mybir.AxisListType.X)
        lns = psmall.tile([1, 1], f32, tag="lnc")
        nc.scalar.activation(lns[:], s_sb[:], Act.Ln)
        res = pscale.tile([1, 1], f32, tag="res")
        nc.vector.tensor_add(res[:], lns[:], L_tile[:])
        nc.sync.dma_start(out_t[:], res[:])
        p_f32 = pscale.tile([128, 4], f32, tag="p_f32")
        nc.vector.tensor_copy(p_f32[:], p_cur[:])
        nc.sync.dma_start(p_out[:], p_f32[:])
        nc.sync.dma_start(L_out[:], L_tile[:])

    nc.finalize()
    return nc


def _host_prep(log_T, log_pi, obs_batch, true_lens, n_steps):
    """Scales, per-core offset tables, initial states, and readout constants."""
    maxlog = float(np.max(log_T))
    ln_S = math.log(128.0) - maxlog  # max fp8 entry = 128

    # e_k ~ -round(log2(S * mean block row-sum)), from a row sample
    sample = np.asarray(log_T[:: max(1, N_STATES // 32), :], dtype=np.float64)
    mean_scaled = float(np.mean(np.exp(sample - maxlog))) * 128.0
    mean_rowsum = mean_scaled * C
    e_k = int(np.clip(-round(math.log2(max(mean_rowsum, 1e-30))), -16, 0))

    offs = np.empty((N_CORES, 128, max(n_steps, 1)), dtype=np.int32)
    v0 = np.empty((N_CORES, 128, 4), dtype=ml_dtypes.bfloat16)
    host_const = np.empty((N_CORES,), dtype=np.float64)
    part = np.arange(128, dtype=np.int32)[:, None]

    for b in range(N_CORES):
        o = np.asarray(obs_batch[b], dtype=np.int64)
        tl = int(true_lens[b])
        blocks = o[:-1] * N_OBS + o[1:]  # step k uses blocks[k-1]
        blocks = blocks[:n_steps].copy()
        blocks[max(tl - 1, 0):] = PAD_BLOCK
        if n_steps == 0:
            blocks = np.array([PAD_BLOCK], dtype=np.int64)
        offs[b] = blocks[None, :].astype(np.int32) * 128 + part

        a0 = np.asarray(log_pi[o[0] * C:(o[0] + 1) * C], dtype=np.float64)
        m0 = float(np.max(a0))
        v0[b] = np.exp(a0 - m0).reshape(4, 128).T.astype(ml_dtypes.bfloat16)
        n_real = min(max(tl - 1, 0), n_steps)  # pad steps contribute nothing
        host_const[b] = m0 - n_real * (ln_S + e_k * math.log(2.0))

    return ln_S, e_k, offs, v0, host_const


def _run(log_T, log_pi, obs_batch, true_lens, n_steps=T - 1,
         renorm_every=6, renorm_defer=3, trace=False, blk_bufs=6,
         repeat_p1=1, repeat_p2=1, n_calls=1):
    from concourse.bass_utils import run_bass_kernel_spmd

    log_T = np.ascontiguousarray(np.asarray(log_T, dtype=np.float32))
    log_pi = np.asarray(log_pi, dtype=np.float32)
    obs_batch = np.asarray(obs_batch)
    true_lens = np.asarray(true_lens)

    ln_S, e_k, offs, v0, host_const = _host_prep(
        log_T, log_pi, obs_batch, true_lens, n_steps)

    nc = _build_bass(n_steps, ln_S, e_k, renorm_every, renorm_defer, blk_bufs,
                     repeat_p1=repeat_p1, repeat_p2=repeat_p2)

    in_maps = [
        {"log_T": log_T, "offs": np.ascontiguousarray(offs[b]),
         "v0": np.ascontiguousarray(v0[b])}
        for b in range(N_CORES)
    ]
    import time as _time
    call_walls = []
    for _ in range(n_calls):
        t0 = _time.time()
        res = run_bass_kernel_spmd(nc, in_maps, core_ids=list(range(N_CORES)),
                                   trace=trace)
        call_walls.append(_time.time() - t0)
    res.call_walls = call_walls
    logZ = np.array(
        [res.results[b]["out"][0, 0] + host_const[b] for b in range(N_CORES)],
        dtype=np.float32,
    )
    return logZ, res


def kernel(log_T, log_pi, obs_batch, true_lens, n_clones=C, **_ignored):
    assert int(n_clones) == C, f"kernel hardcodes n_clones={C}, got {n_clones}"
    logZ, _ = _run(log_T, log_pi, obs_batch, true_lens)
    return logZ



# revision 2
# speedup vs baseline: 1.4090x; 1.4090x over previous
"""Trainium2 Bass kernel for the CSCG batched masked HMM forward pass.

Problem: for each of B=8 padded observation sequences, run a log-space HMM
forward recurrence restricted to 512-state clone blocks selected by
consecutive observation pairs, and read log P(obs) at true_len-1.

Strategy (one sequence per NeuronCore, 8 cores):
  * Work in linear space with exact power-of-two step scaling instead of
    logsumexp: the fp8 table holds f*exp(log_T) (f chosen so the max entry
    sits near 128 and f = 16*2^e_k exactly), and each step multiplies by
    2^-e_k, making the mean per-step growth exactly 1 in expectation - no
    on-device renormalization is needed for 1023 steps (stochastic drift
    is a few nats; bf16 has ~e^+-88 of headroom).
  * The host precomputes the fp8 block-major table once (256 blocks of
    512x512 -> 128 rows x 2KB each, plus one constant pad block), so the
    device never touches the f32 log_T.
  * Per step, one HWDGE dma_start with a register-sourced dynamic DRAM
    offset fetches the contiguous 256KB block for the observation pair -
    no gpsimd SWDGE descriptor generation on the critical path.
  * The 512x512 matvec runs as 16 PSUM-accumulated (K=128, M=128, N=1)
    matmuls with fp8 weights (FWL fast weight load) whose input/output
    layout is identical ([128 partitions = low 7 bits of state, 4 free =
    high 2 bits]), so the serial chain needs no transposes. The psum->SBUF
    copies are per-column, split across DVE and ACT, so the next step's
    matmuls unblock incrementally.
  * Steps past true_len-1 multiply by a constant pad block that preserves
    sum(v) exactly, so all cores run a uniform step count and the final
    readout log(sum(v)) equals the value at true_len-1 (up to host-side
    constants).
"""

import math
from contextlib import ExitStack

import numpy as np
import ml_dtypes

N_OBS = 16
C = 512
N_STATES = N_OBS * C  # 8192
B = 8
T = 1024
N_CORES = 8
PAD_BLOCK = N_OBS * N_OBS  # index of the constant pad block
N_TABLE_ROWS = (PAD_BLOCK + 1) * 128  # 33024 rows of 2048 bytes


def _build_bass(n_steps: int, k_copy: float, blk_bufs: int = 8):
    import concourse.bass as bass
    import concourse.tile as tile
    from concourse import bacc, mybir

    fp8 = mybir.dt.float8e4
    bf16 = mybir.dt.bfloat16
    f32 = mybir.dt.float32
    i32 = mybir.dt.int32
    SP = mybir.EngineType.SP

    nc = bacc.Bacc(None, target_bir_lowering=False)
    table_in = nc.dram_tensor("table", [N_TABLE_ROWS, 2048], fp8,
                              kind="ExternalInput")
    offs_in = nc.dram_tensor("offs", [1, max(n_steps, 1)], i32,
                             kind="ExternalInput")
    v0_in = nc.dram_tensor("v0", [128, 4], bf16, kind="ExternalInput")
    p_out = nc.dram_tensor("p_out", [128, 4], f32, kind="ExternalOutput")

    with ExitStack() as ctx:
        tc = ctx.enter_context(tile.TileContext(nc))

        pconst = ctx.enter_context(tc.tile_pool(name="pconst", bufs=1))
        pblk = ctx.enter_context(tc.tile_pool(name="pblk", bufs=blk_bufs))
        pp = ctx.enter_context(tc.tile_pool(name="pp", bufs=3))
        pfin = ctx.enter_context(tc.tile_pool(name="pfin", bufs=1))
        ps_v = ctx.enter_context(tc.tile_pool(name="ps_v", bufs=2,
                                              space="PSUM"))

        offs_sb = pconst.tile([1, max(n_steps, 1)], i32)
        nc.sync.dma_start(offs_sb[:], offs_in[:])

        p_cur = pp.tile([128, 4], bf16, tag="p")
        nc.sync.dma_start(p_cur[:], v0_in[:])

        for k in range(n_steps):
            off_val = nc.values_load(
                offs_sb[0:1, k:k + 1], engines=[SP],
                min_val=0, max_val=(N_TABLE_ROWS - 128),
                skip_runtime_bounds_check=True)
            blk = pblk.tile([128, 2048], fp8, tag="blk")
            nc.sync.dma_start(blk[:], table_in[bass.ds(off_val, 128), :])

            psum = ps_v.tile([128, 4], f32, tag="v")
            for j in range(4):
                for step_i, i in enumerate(range(4)):
                    nc.tensor.matmul(
                        out=psum[:, j:j + 1],
                        lhsT=blk[:, i * 512 + j * 128:
                                 i * 512 + (j + 1) * 128],
                        rhs=p_cur[:, i:i + 1],
                        start=(step_i == 0),
                        stop=(step_i == 3),
                    )

            p_next = pp.tile([128, 4], bf16, tag="p")
            for c in range(4):
                if c % 2 == 0:
                    nc.vector.tensor_scalar_mul(
                        p_next[:, c:c + 1], psum[:, c:c + 1], k_copy)
                else:
                    nc.scalar.mul(
                        p_next[:, c:c + 1], psum[:, c:c + 1], k_copy)
            p_cur = p_next

        p_f32 = pfin.tile([128, 4], f32)
        nc.vector.tensor_copy(p_f32[:], p_cur[:])
        nc.sync.dma_start(p_out[:], p_f32[:])

    nc.finalize()
    return nc


def _host_prep(log_T, log_pi, obs_batch, true_lens, n_steps):
    """fp8 table, per-core step-offset tables, initial states, constants."""
    fp8_np = ml_dtypes.float8_e4m3

    maxlog = float(np.max(log_T))
    M = math.exp(maxlog)
    # f = 16 * 2^e_k with f*M near 128 => max table entry in [90, 181]
    e_k = int(round(math.log2(128.0 / M) - 4.0))
    assert 0 <= e_k - 9 <= 7, f"pad entry 2^{e_k - 9} not fp8-exact"
    ln_f = math.log(16.0) + e_k * math.log(2.0)
    k_copy = 2.0 ** (-e_k)
    kappa = 2.0 ** (e_k - 9)  # pad entry: rowsum 512*kappa = 2^e_k exactly

    # Block-major fp8 table: row (op*16+oc)*128 + i_lo, col i_hi*512 + j
    # holds f*exp(log_T)[op*512 + i_hi*128 + i_lo, oc*512 + j].
    table = np.empty((N_TABLE_ROWS, 2048), dtype=fp8_np)
    lt = np.asarray(log_T, dtype=np.float32)
    for op in range(N_OBS):
        rows = lt[op * C:(op + 1) * C, :]  # [512, 8192]
        e8 = np.exp(rows + np.float32(ln_f)).astype(fp8_np)
        # [i_hi, i_lo, oc, j] -> [oc, i_lo, i_hi, j]
        e6 = e8.reshape(4, 128, N_OBS, 512).transpose(2, 1, 0, 3)
        table[op * N_OBS * 128:(op + 1) * N_OBS * 128, :] = \
            e6.reshape(N_OBS * 128, 2048)
    table[PAD_BLOCK * 128:, :] = fp8_np(kappa)

    offs = np.empty((N_CORES, 1, max(n_steps, 1)), dtype=np.int32)
    v0 = np.empty((N_CORES, 128, 4), dtype=ml_dtypes.bfloat16)
    host_const = np.empty((N_CORES,), dtype=np.float64)

    for b in range(N_CORES):
        o = np.asarray(obs_batch[b], dtype=np.int64)
        tl = int(true_lens[b])
        blocks = o[:-1] * N_OBS + o[1:]  # step k uses blocks[k]
        blocks = blocks[:n_steps].copy()
        blocks[max(tl - 1, 0):] = PAD_BLOCK
        if n_steps == 0:
            blocks = np.array([PAD_BLOCK], dtype=np.int64)
        offs[b, 0, :] = (blocks * 128).astype(np.int32)

        a0 = np.asarray(log_pi[o[0] * C:(o[0] + 1) * C], dtype=np.float64)
        m0 = float(np.max(a0))
        v0[b] = np.exp(a0 - m0).reshape(4, 128).T.astype(ml_dtypes.bfloat16)
        n_real = min(max(tl - 1, 0), n_steps)  # pad steps preserve sum(v)
        # per real step the kernel multiplies by f*exp(.)*2^-e_k = 16*exp(.)
        host_const[b] = m0 - n_real * math.log(16.0)

    return k_copy, table, offs, v0, host_const


def _run(log_T, log_pi, obs_batch, true_lens, n_steps=None,
         trace=False, blk_bufs=8, **_ignored):
    from concourse.bass_utils import run_bass_kernel_spmd

    log_pi = np.asarray(log_pi, dtype=np.float32)
    obs_batch = np.asarray(obs_batch)
    true_lens = np.asarray(true_lens)
    if n_steps is None:
        n_steps = max(int(np.max(true_lens)) - 1, 0)

    k_copy, table, offs, v0, host_const = _host_prep(
        log_T, log_pi, obs_batch, true_lens, n_steps)

    nc = _build_bass(n_steps, k_copy, blk_bufs)

    in_maps = [
        {"table": table, "offs": np.ascontiguousarray(offs[b]),
         "v0": np.ascontiguousarray(v0[b])}
        for b in range(N_CORES)
    ]
    res = run_bass_kernel_spmd(nc, in_maps, core_ids=list(range(N_CORES)),
                               trace=trace)
    logZ = np.empty((N_CORES,), dtype=np.float32)
    for b in range(N_CORES):
        p = res.results[b]["p_out"].astype(np.float64)
        logZ[b] = math.log(float(p.sum())) + host_const[b]
    return logZ, res


def kernel(log_T, log_pi, obs_batch, true_lens, n_clones=C, **_ignored):
    assert int(n_clones) == C, f"kernel hardcodes n_clones={C}, got {n_clones}"
    logZ, _ = _run(log_T, log_pi, obs_batch, true_lens)
    return logZ
